# revision 9
# baseline (speedup 1.0000x reference)
"""Longformer sliding-window attention (W=128) on 8 Trainium2 NeuronCores.

Strategy (fp16 compute, f32 softmax stats, batch*head sharding):
  - 24 (b,h) slices across 8 cores, 3 per core; window attention is local.
  - Host pre-casts q/k/v to fp16 and pre-transposes q/k to [d, seq] (with
    two bh slices packed along d to fill 128 SBUF partitions), so the
    device sees DMA-friendly layouts and half the input bytes.
  - Per (bh, query block n of 128):
      PE : mask matmul (identity @ maskbias, PSUM start) then QK matmul
           (lhsT=qT[64,128] fp16, rhs=kT[64,384] fp16) accumulating into
           the same PSUM bank -> masked scores f32
      ACT: em = exp(scale*scores - 6) fp16 (uniform -6 shift cancels in
           softmax, keeps exp in fp16 range), accum_out = row sums f32
      DVE: r = 1/sums;  em_n = em * r  -> normalized prob rows [128, 384]
      PE : transpose em_n 128-chunks -> PSUM, DVE copy -> SBUF, then
           ctx[128,64] = sum_c emT_c.T @ v_c accumulated in PSUM f32
  - Device writes em_n blocks [128, NB, 384] fp16 per bh; the host
    extracts the diagonal 257-band (probs[x, t] = em_n[n, x, x+t]) and
    upcasts to f32. End-to-end abs-rel error ~5e-4 vs the f32 reference.
"""

import numpy as np

import concourse.bacc as bacc
import concourse.mybir as mybir
import concourse.tile as tile
from concourse.bass_utils import run_bass_kernel_spmd

F16 = mybir.dt.float16
F32 = mybir.dt.float32

NCORES = 8
B, H, S, D = 2, 12, 4096, 64
W = 128
NB = S // W          # 32 query blocks per bh
T = 2 * W + 1        # 257 band width
BHPC = (B * H) // NCORES  # 3 bh slices per core
NPAIR = (BHPC + 1) // 2   # bh pairs packed along d into 128 partitions
NEG = -30720.0
SCALE = 0.125        # 1/sqrt(D)
EBIAS = -6.0         # uniform exp shift, cancels in softmax


def _np_constants():
    x = np.arange(W)[:, None]
    y = np.arange(3 * W)[None, :]
    band = (y - x >= 0) & (y - x <= 2 * W)
    first = band & (y >= W)
    last = band & (y < 2 * W)
    masks = np.stack(
        [np.where(m, 0.0, NEG) for m in (first, band, last)]
    ).astype(np.float16)  # [3, 128, 384]
    ident = np.eye(W, dtype=np.float16)
    ebias = np.full((W, 1), EBIAS, dtype=np.float32)
    return masks, ident, ebias


def _build_program(BHPC=BHPC, NB=NB):
    S = NB * W
    NPAIR = (BHPC + 1) // 2
    nc = bacc.Bacc("TRN2", target_bir_lowering=False, debug=False)

    # host-prepared fp16 inputs: qt/kt are [d, seq] transposed, two bh
    # packed along d; v is in natural [seq, d] layout
    qt_i = nc.dram_tensor("qt", [NPAIR, 2 * D, S], F16, kind="ExternalInput").ap()
    kt_i = nc.dram_tensor("kt", [NPAIR, 2 * D, S], F16, kind="ExternalInput").ap()
    v_i = nc.dram_tensor("v", [BHPC, S, D], F16, kind="ExternalInput").ap()
    ctx_o = nc.dram_tensor("ctx", [BHPC, S, D], F16, kind="ExternalOutput").ap()
    # full normalized score rows; host extracts the 257 diagonal band
    em_o = nc.dram_tensor("emn", [BHPC, S, 3 * W], F16, kind="ExternalOutput").ap()

    masks_np, ident_np, ebias_np = _np_constants()
    masks_d = nc.inline_tensor(masks_np, "masks_c").ap()
    ident_d = nc.inline_tensor(ident_np, "ident_c").ap()
    ebias_d = nc.inline_tensor(ebias_np, "ebias_c").ap()

    with tile.TileContext(nc) as tc:
        with (
            tc.tile_pool(name="const", bufs=1) as constp,
            tc.tile_pool(name="qt", bufs=2) as qtp,
            tc.tile_pool(name="kt", bufs=2) as ktp,
            tc.tile_pool(name="vp", bufs=2) as vp,
            tc.tile_pool(name="em", bufs=3) as emp,
            tc.tile_pool(name="emn", bufs=1) as emnp,
            tc.tile_pool(name="emt", bufs=6) as emtp,
            tc.tile_pool(name="stat", bufs=8) as statp,
            tc.tile_pool(name="ctxs", bufs=1) as ctxsp,
            tc.tile_pool(name="ps_s", bufs=2, space="PSUM") as ps_s,
            tc.tile_pool(name="ps_t", bufs=3, space="PSUM") as ps_t,
            tc.tile_pool(name="ps_c", bufs=2, space="PSUM") as ps_c,
        ):
            masks = constp.tile([W, 3, 3 * W], F16, tag="masks")
            nc.sync.dma_start(masks[:], masks_d.rearrange("m x y -> x m y"))
            ident = constp.tile([W, W], F16, tag="ident")
            nc.sync.dma_start(ident[:], ident_d)
            ebias = constp.tile([W, 1], F32, tag="ebias")
            nc.sync.dma_start(ebias[:], ebias_d)

            emn_all = emnp.tile([W, BHPC, NB, 3 * W], F16, tag="emn")
            ctx_all = ctxsp.tile([W, BHPC, NB, D], F16, tag="ctxs")

            qT = {}
            kT = {}

            def load_pair(p):
                qT[p] = qtp.tile([2 * D, S], F16, tag="qt", name=f"qT{p}")
                nc.sync.dma_start(qT[p][:], qt_i[p])
                kT[p] = ktp.tile([2 * D, S], F16, tag="kt", name=f"kT{p}")
                nc.sync.dma_start(kT[p][:], kt_i[p])

            for bh in range(BHPC):
                p, half = bh // 2, bh % 2
                if half == 0:
                    load_pair(p)
                dlo = half * D

                v_sb = vp.tile([W, NB, D], F16, tag="v")
                nc.sync.dma_start(
                    v_sb[:], v_i[bh].rearrange("(n x) d -> x n d", x=W)
                )

                for n in range(NB):
                    c_lo = 1 if n == 0 else 0
                    c_hi = 2 if n == NB - 1 else 3
                    mv = 0 if n == 0 else (2 if n == NB - 1 else 1)

                    psum_s = ps_s.tile([W, 3 * W], F32, tag="ps_s")
                    nc.tensor.matmul(
                        psum_s[:], ident[:], masks[:, mv, :],
                        start=True, stop=False,
                    )
                    nc.tensor.matmul(
                        psum_s[:, c_lo * W : c_hi * W],
                        qT[p][dlo : dlo + D, n * W : (n + 1) * W],
                        kT[p][dlo : dlo + D, (n - 1 + c_lo) * W : (n - 1 + c_hi) * W],
                        start=False, stop=True,
                    )

                    em = emp.tile([W, 3 * W], F16, tag="em")
                    ssum = statp.tile([W, 1], F32, tag="ssum")
                    nc.scalar.activation(
                        em[:], psum_s[:], mybir.ActivationFunctionType.Exp,
                        bias=ebias[:], scale=SCALE, accum_out=ssum[:],
                    )
                    r = statp.tile([W, 1], F32, tag="r")
                    nc.vector.reciprocal(r[:], ssum[:])
                    em_n = emn_all[:, bh, n]
                    nc.vector.tensor_scalar_mul(em_n[:], em[:], r[:])

                    psum_ctx = ps_c.tile([W, D], F32, tag="ps_c")
                    for c in range(c_lo, c_hi):
                        pt = ps_t.tile([W, W], F16, tag="ps_t")
                        nc.tensor.transpose(
                            pt[:], em_n[:, c * W : (c + 1) * W], ident[:]
                        )
                        emT = emtp.tile([W, W], F16, tag="emt")
                        nc.vector.tensor_copy(emT[:], pt[:])
                        nc.tensor.matmul(
                            psum_ctx[:], emT[:], v_sb[:, n - 1 + c, :],
                            start=(c == c_lo), stop=(c == c_hi - 1),
                        )
                    nc.vector.tensor_copy(ctx_all[:, bh, n, :], psum_ctx[:])

                nc.sync.dma_start(
                    ctx_o[bh].rearrange("(n x) d -> x n d", x=W),
                    ctx_all[:, bh],
                )
                nc.sync.dma_start(
                    em_o[bh].rearrange("(n x) y -> x n y", x=W),
                    emn_all[:, bh],
                )

    nc.compile()
    return nc


_PROGRAM = None


def _get_program():
    global _PROGRAM
    if _PROGRAM is None:
        _PROGRAM = _build_program()
    return _PROGRAM


# host-side diagonal band extraction index: band[x, t] = em_row[x, x + t]
_BAND_IDX = (np.arange(W)[:, None] + np.arange(T)[None, :])  # [128, 257]


def _extract_band(em_raw):
    """[nbh, S, 384] fp16 block rows -> [nbh, S, 257] f32 band."""
    nbh, s, _ = em_raw.shape
    nb = s // W
    em4 = em_raw.reshape(nbh, nb, W, 3 * W)
    idx = _BAND_IDX[None, None].astype(np.intp)
    band = np.take_along_axis(em4, np.broadcast_to(idx, (nbh, nb, W, T)), axis=3)
    return band.reshape(nbh, s, T)


def _prep_core_inputs(qf, kf, vf, lo):
    """Host-side fp16 cast + [d, seq] transpose + bh-pair packing."""
    q16 = qf[lo : lo + BHPC].astype(np.float16)
    k16 = kf[lo : lo + BHPC].astype(np.float16)
    v16 = vf[lo : lo + BHPC].astype(np.float16)
    qt = np.zeros((NPAIR, 2 * D, S), np.float16)
    kt = np.zeros((NPAIR, 2 * D, S), np.float16)
    for bh in range(BHPC):
        p, half = bh // 2, bh % 2
        qt[p, half * D : (half + 1) * D] = q16[bh].T
        kt[p, half * D : (half + 1) * D] = k16[bh].T
    return {
        "qt": np.ascontiguousarray(qt),
        "kt": np.ascontiguousarray(kt),
        "v": np.ascontiguousarray(v16),
    }


def kernel(q, k, v, numeric_embedding_manager=None, **_unused):
    nc = _get_program()
    qf = np.asarray(q, dtype=np.float32).reshape(B * H, S, D)
    kf = np.asarray(k, dtype=np.float32).reshape(B * H, S, D)
    vf = np.asarray(v, dtype=np.float32).reshape(B * H, S, D)

    in_maps = [
        _prep_core_inputs(qf, kf, vf, i * BHPC) for i in range(NCORES)
    ]
    res = run_bass_kernel_spmd(nc, in_maps, core_ids=list(range(NCORES)))

    ctx = np.concatenate(
        [res.results[i]["ctx"] for i in range(NCORES)], axis=0
    ).astype(np.float32).reshape(B, H, S, D)
    em_raw = np.concatenate(
        [res.results[i]["emn"] for i in range(NCORES)], axis=0
    )
    probs = _extract_band(em_raw).astype(np.float32)
    return ctx, probs


# revision 13
# speedup vs baseline: 1.0023x; 1.0023x over previous
"""Longformer sliding-window attention (W=128) on 8 Trainium2 NeuronCores.

Strategy (fp16 compute, f32 softmax stats, batch*head sharding):
  - 24 (b,h) slices across 8 cores, 3 per core; window attention is local.
  - Host pre-casts q/k/v to fp16 and pre-transposes q/k to [d, seq] (with
    two bh slices packed along d to fill 128 SBUF partitions), so the
    device sees DMA-friendly layouts and half the input bytes.
  - Per (bh, query block n of 128):
      PE : mask matmul (identity @ maskbias, PSUM start) then QK matmul
           (lhsT=qT[64,128] fp16, rhs=kT[64,384] fp16) accumulating into
           the same PSUM bank -> masked scores f32
      ACT: em = exp(scale*scores - 6) fp16 (uniform -6 shift cancels in
           softmax, keeps exp in fp16 range), accum_out = row sums f32
      DVE: r = 1/sums;  em_n = em * r  -> normalized prob rows [128, 384]
      PE : transpose em_n 128-chunks -> PSUM, DVE copy -> SBUF, then
           ctx[128,64] = sum_c emT_c.T @ v_c accumulated in PSUM f32
  - Device writes em_n blocks [128, NB, 384] fp16 per bh; the host
    extracts the diagonal 257-band (probs[x, t] = em_n[n, x, x+t]) and
    upcasts to f32. End-to-end abs-rel error ~5e-4 vs the f32 reference.
"""

import numpy as np

import concourse.bacc as bacc
import concourse.mybir as mybir
import concourse.tile as tile
from concourse.bass_utils import run_bass_kernel_spmd

F16 = mybir.dt.float16
F32 = mybir.dt.float32

NCORES = 8
B, H, S, D = 2, 12, 4096, 64
W = 128
NB = S // W          # 32 query blocks per bh
T = 2 * W + 1        # 257 band width
BHPC = (B * H) // NCORES  # 3 bh slices per core
NPAIR = (BHPC + 1) // 2   # bh pairs packed along d into 128 partitions
NEG = -30720.0
SCALE = 0.125        # 1/sqrt(D)
EBIAS = -6.0         # uniform exp shift, cancels in softmax


def _np_constants():
    x = np.arange(W)[:, None]
    y = np.arange(3 * W)[None, :]
    band = (y - x >= 0) & (y - x <= 2 * W)
    first = band & (y >= W)
    last = band & (y < 2 * W)
    masks = np.stack(
        [np.where(m, 0.0, NEG) for m in (first, band, last)]
    ).astype(np.float16)  # [3, 128, 384]
    ident = np.eye(W, dtype=np.float16)
    ebias = np.full((W, 1), EBIAS, dtype=np.float32)
    return masks, ident, ebias


def _build_program(BHPC=BHPC, NB=NB):
    S = NB * W
    NPAIR = (BHPC + 1) // 2
    nc = bacc.Bacc("TRN2", target_bir_lowering=False, debug=False)

    # host-prepared fp16 inputs: qt/kt are [d, seq] transposed, two bh
    # packed along d; v is in natural [seq, d] layout
    qt_i = nc.dram_tensor("qt", [NPAIR, 2 * D, S], F16, kind="ExternalInput").ap()
    kt_i = nc.dram_tensor("kt", [NPAIR, 2 * D, S], F16, kind="ExternalInput").ap()
    v_i = nc.dram_tensor("v", [BHPC, S, D], F16, kind="ExternalInput").ap()
    ctx_o = nc.dram_tensor("ctx", [BHPC, S, D], F16, kind="ExternalOutput").ap()
    # full normalized score rows; host extracts the 257 diagonal band
    em_o = nc.dram_tensor("emn", [BHPC, S, 3 * W], F16, kind="ExternalOutput").ap()

    masks_np, ident_np, ebias_np = _np_constants()
    masks_d = nc.inline_tensor(masks_np, "masks_c").ap()
    ident_d = nc.inline_tensor(ident_np, "ident_c").ap()
    ebias_d = nc.inline_tensor(ebias_np, "ebias_c").ap()

    with tile.TileContext(nc) as tc:
        with (
            tc.tile_pool(name="const", bufs=1) as constp,
            tc.tile_pool(name="qt", bufs=2) as qtp,
            tc.tile_pool(name="kt", bufs=2) as ktp,
            tc.tile_pool(name="vp", bufs=2) as vp,
            tc.tile_pool(name="em", bufs=3) as emp,
            tc.tile_pool(name="emn", bufs=1) as emnp,
            tc.tile_pool(name="emt", bufs=6) as emtp,
            tc.tile_pool(name="stat", bufs=8) as statp,
            tc.tile_pool(name="ctxs", bufs=1) as ctxsp,
            tc.tile_pool(name="ps_s", bufs=2, space="PSUM") as ps_s,
            tc.tile_pool(name="ps_t", bufs=3, space="PSUM") as ps_t,
            tc.tile_pool(name="ps_c", bufs=2, space="PSUM") as ps_c,
        ):
            masks = constp.tile([W, 3, 3 * W], F16, tag="masks")
            nc.sync.dma_start(masks[:], masks_d.rearrange("m x y -> x m y"))
            ident = constp.tile([W, W], F16, tag="ident")
            nc.sync.dma_start(ident[:], ident_d)
            ebias = constp.tile([W, 1], F32, tag="ebias")
            nc.sync.dma_start(ebias[:], ebias_d)

            emn_all = emnp.tile([W, BHPC, NB, 3 * W], F16, tag="emn")
            ctx_all = ctxsp.tile([W, BHPC, NB, D], F16, tag="ctxs")

            # ~4us of dummy matmuls to push the PE HAM clock gate to 8/8
            # before the real work arrives
            warm_ps = ps_s.tile([W, 3 * W], F32, tag="ps_s", name="warm_ps")
            for wi in range(12):
                nc.tensor.matmul(
                    warm_ps[:], ident[:], masks[:, wi % 3, :],
                    start=(wi == 0), stop=(wi == 11),
                )

            qT = {}
            kT = {}

            def load_pair(p):
                qT[p] = qtp.tile([2 * D, S], F16, tag="qt", name=f"qT{p}")
                nc.sync.dma_start(qT[p][:], qt_i[p])
                kT[p] = ktp.tile([2 * D, S], F16, tag="kt", name=f"kT{p}")
                nc.sync.dma_start(kT[p][:], kt_i[p])

            for bh in range(BHPC):
                p, half = bh // 2, bh % 2
                if half == 0:
                    load_pair(p)
                dlo = half * D

                v_sb = vp.tile([W, NB, D], F16, tag="v")
                nc.sync.dma_start(
                    v_sb[:], v_i[bh].rearrange("(n x) d -> x n d", x=W)
                )

                for n in range(NB):
                    c_lo = 1 if n == 0 else 0
                    c_hi = 2 if n == NB - 1 else 3
                    mv = 0 if n == 0 else (2 if n == NB - 1 else 1)

                    psum_s = ps_s.tile([W, 3 * W], F32, tag="ps_s")
                    qk_args = (
                        psum_s[:, c_lo * W : c_hi * W],
                        qT[p][dlo : dlo + D, n * W : (n + 1) * W],
                        kT[p][dlo : dlo + D, (n - 1 + c_lo) * W : (n - 1 + c_hi) * W],
                    )
                    nc.tensor.matmul(
                        psum_s[:], ident[:], masks[:, mv, :],
                        start=True, stop=False,
                    )
                    nc.tensor.matmul(*qk_args, start=False, stop=True)

                    em = emp.tile([W, 3 * W], F16, tag="em")
                    ssum = statp.tile([W, 1], F32, tag="ssum")
                    nc.scalar.activation(
                        em[:], psum_s[:], mybir.ActivationFunctionType.Exp,
                        bias=ebias[:], scale=SCALE, accum_out=ssum[:],
                    )
                    r = statp.tile([W, 1], F32, tag="r")
                    nc.vector.reciprocal(r[:], ssum[:])
                    em_n = emn_all[:, bh, n]
                    nc.vector.tensor_scalar_mul(em_n[:], em[:], r[:])

                    psum_ctx = ps_c.tile([W, D], F32, tag="ps_c")
                    for c in range(c_lo, c_hi):
                        pt = ps_t.tile([W, W], F16, tag="ps_t")
                        nc.tensor.transpose(
                            pt[:], em_n[:, c * W : (c + 1) * W], ident[:]
                        )
                        emT = emtp.tile([W, W], F16, tag="emt")
                        nc.vector.tensor_copy(emT[:], pt[:])
                        nc.tensor.matmul(
                            psum_ctx[:], emT[:], v_sb[:, n - 1 + c, :],
                            start=(c == c_lo), stop=(c == c_hi - 1),
                        )
                    nc.vector.tensor_copy(ctx_all[:, bh, n, :], psum_ctx[:])

                nc.sync.dma_start(
                    ctx_o[bh].rearrange("(n x) d -> x n d", x=W),
                    ctx_all[:, bh],
                )
                nc.sync.dma_start(
                    em_o[bh].rearrange("(n x) y -> x n y", x=W),
                    emn_all[:, bh],
                )

    nc.compile()
    return nc


_PROGRAM = None


def _get_program():
    global _PROGRAM
    if _PROGRAM is None:
        _PROGRAM = _build_program()
    return _PROGRAM


# host-side diagonal band extraction index: band[x, t] = em_row[x, x + t]
_BAND_IDX = (np.arange(W)[:, None] + np.arange(T)[None, :])  # [128, 257]


def _extract_band(em_raw):
    """[nbh, S, 384] fp16 block rows -> [nbh, S, 257] f32 band."""
    nbh, s, _ = em_raw.shape
    nb = s // W
    em4 = em_raw.reshape(nbh, nb, W, 3 * W)
    idx = _BAND_IDX[None, None].astype(np.intp)
    band = np.take_along_axis(em4, np.broadcast_to(idx, (nbh, nb, W, T)), axis=3)
    return band.reshape(nbh, s, T)


def _prep_core_inputs(qf, kf, vf, lo):
    """Host-side fp16 cast + [d, seq] transpose + bh-pair packing."""
    q16 = qf[lo : lo + BHPC].astype(np.float16)
    k16 = kf[lo : lo + BHPC].astype(np.float16)
    v16 = vf[lo : lo + BHPC].astype(np.float16)
    qt = np.zeros((NPAIR, 2 * D, S), np.float16)
    kt = np.zeros((NPAIR, 2 * D, S), np.float16)
    for bh in range(BHPC):
        p, half = bh // 2, bh % 2
        qt[p, half * D : (half + 1) * D] = q16[bh].T
        kt[p, half * D : (half + 1) * D] = k16[bh].T
    return {
        "qt": np.ascontiguousarray(qt),
        "kt": np.ascontiguousarray(kt),
        "v": np.ascontiguousarray(v16),
    }


def kernel(q, k, v, numeric_embedding_manager=None, **_unused):
    nc = _get_program()
    qf = np.asarray(q, dtype=np.float32).reshape(B * H, S, D)
    kf = np.asarray(k, dtype=np.float32).reshape(B * H, S, D)
    vf = np.asarray(v, dtype=np.float32).reshape(B * H, S, D)

    in_maps = [
        _prep_core_inputs(qf, kf, vf, i * BHPC) for i in range(NCORES)
    ]
    res = run_bass_kernel_spmd(nc, in_maps, core_ids=list(range(NCORES)))

    ctx = np.concatenate(
        [res.results[i]["ctx"] for i in range(NCORES)], axis=0
    ).astype(np.float32).reshape(B, H, S, D)
    em_raw = np.concatenate(
        [res.results[i]["emn"] for i in range(NCORES)], axis=0
    )
    probs = _extract_band(em_raw).astype(np.float32)
    return ctx, probs


# revision 29
# speedup vs baseline: 1.3026x; 1.2996x over previous
"""Longformer sliding-window attention (W=128) on 8 Trainium2 NeuronCores.

Strategy (fp16 compute, f32 softmax stats, batch*head sharding):
  - 24 (b,h) slices across 8 cores, 3 per core; window attention is local.
  - Host pre-casts q/k/v to fp16 and pre-transposes q/k to [d, seq] (with
    two bh slices packed along d to fill 128 SBUF partitions), so the
    device sees DMA-friendly layouts and half the input bytes.
  - Per (bh, query block n of 128):
      PE : mask matmul (identity @ maskbias, PSUM start) then QK matmul
           (lhsT=qT[64,128] fp16, rhs=kT[64,384] fp16) accumulating into
           the same PSUM bank -> masked scores f32
      ACT: em = exp(scale*scores - 6) fp16 (uniform -6 shift cancels in
           softmax, keeps exp in fp16 range), accum_out = row sums f32
      DVE: r = 1/sums;  em_n = em * r  -> normalized prob rows [128, 384]
      PE : transpose em_n 128-chunks -> PSUM, DVE copy -> SBUF, then
           ctx[128,64] = sum_c emT_c.T @ v_c accumulated in PSUM f32
  - Device writes em_n blocks [128, NB, 384] fp16 per bh; the host
    extracts the diagonal 257-band (probs[x, t] = em_n[n, x, x+t]) and
    upcasts to f32. End-to-end abs-rel error ~5e-4 vs the f32 reference.
"""

import numpy as np

import concourse.bacc as bacc
import concourse.mybir as mybir
import concourse.tile as tile
from concourse.bass_utils import run_bass_kernel_spmd

F16 = mybir.dt.float16
F32 = mybir.dt.float32
U8 = mybir.dt.uint8

NCORES = 8
B, H, S, D = 2, 12, 4096, 64
W = 128
NB = S // W          # 32 query blocks per bh
T = 2 * W + 1        # 257 band width
BHPC = (B * H) // NCORES  # 3 bh slices per core
NPAIR = (BHPC + 1) // 2   # bh pairs packed along d into 128 partitions
NEG = -30720.0
SCALE = 0.125        # 1/sqrt(D)
EBIAS = -6.0         # uniform exp shift, cancels in softmax


def _np_constants():
    x = np.arange(W)[:, None]
    y = np.arange(3 * W)[None, :]
    band = (y - x >= 0) & (y - x <= 2 * W)
    first = band & (y >= W)
    last = band & (y < 2 * W)
    # additive -30720 masks for the two edge blocks (define full PSUM rows)
    masks = np.stack(
        [np.where(m, 0.0, NEG) for m in (first, last)]
    ).astype(np.float16)  # [2, 128, 384]
    # transposed band-validity mask for the predicated emT copies:
    # emT layout is [y_local, c*W + x] = em[x, c*W + y_local]
    yl = np.arange(W)[:, None]
    xx = np.arange(W)[None, :]
    mt = np.zeros((W, 3 * W), np.uint8)
    for c in range(3):
        yg = c * W + yl
        mt[:, c * W : (c + 1) * W] = ((yg - xx >= 0) & (yg - xx <= 2 * W))
    ident = np.eye(W, dtype=np.float16)
    ebias = np.full((W, 1), EBIAS, dtype=np.float32)
    return masks, mt, ident, ebias


def _build_program(BHPC=BHPC, NB=NB):
    S = NB * W
    NPAIR = (BHPC + 1) // 2
    nc = bacc.Bacc("TRN2", target_bir_lowering=False, debug=False)

    # host-prepared fp16 inputs: qt/kt are [d, seq] transposed, two bh
    # packed along d; v is in natural [seq, d] layout
    qt_i = nc.dram_tensor("qt", [NPAIR, 2 * D, S], F16, kind="ExternalInput").ap()
    kt_i = nc.dram_tensor("kt", [NPAIR, 2 * D, S], F16, kind="ExternalInput").ap()
    v_i = nc.dram_tensor("v", [BHPC, S, D], F16, kind="ExternalInput").ap()
    # unnormalized outputs; host extracts the 257 diagonal band, computes
    # row sums over it, and normalizes both probs and ctx
    ctx_o = nc.dram_tensor("ctx", [BHPC, S, D], F16, kind="ExternalOutput").ap()
    em_o = nc.dram_tensor("emn", [BHPC, S, 3 * W], F16, kind="ExternalOutput").ap()

    masks_np, mt_np, ident_np, ebias_np = _np_constants()
    masks_d = nc.inline_tensor(masks_np, "masks_c").ap()
    mt_d = nc.inline_tensor(mt_np, "mt_c").ap()
    ident_d = nc.inline_tensor(ident_np, "ident_c").ap()
    ebias_d = nc.inline_tensor(ebias_np, "ebias_c").ap()

    with tile.TileContext(nc) as tc:
        with (
            tc.tile_pool(name="const", bufs=1) as constp,
            tc.tile_pool(name="qt", bufs=2) as qtp,
            tc.tile_pool(name="kt", bufs=2) as ktp,
            tc.tile_pool(name="vp", bufs=2) as vp,
            tc.tile_pool(name="emn", bufs=1) as emnp,
            tc.tile_pool(name="emt", bufs=2) as emtp,
            tc.tile_pool(name="ctxs", bufs=1) as ctxsp,
            tc.tile_pool(name="ps_s", bufs=2, space="PSUM") as ps_s,
            tc.tile_pool(name="ps_t", bufs=3, space="PSUM") as ps_t,
            tc.tile_pool(name="ps_c", bufs=2, space="PSUM") as ps_c,
        ):
            masks = constp.tile([W, 2, 3 * W], F16, tag="masks")
            nc.sync.dma_start(masks[:], masks_d.rearrange("m x y -> x m y"))
            maskT = constp.tile([W, 3 * W], U8, tag="maskT")
            nc.sync.dma_start(maskT[:], mt_d)
            ident = constp.tile([W, W], F16, tag="ident")
            nc.sync.dma_start(ident[:], ident_d)
            ebias = constp.tile([W, 1], F32, tag="ebias")
            nc.sync.dma_start(ebias[:], ebias_d)

            emn_all = emnp.tile([W, BHPC, NB, 3 * W], F16, tag="emn")
            ctx_all = ctxsp.tile([W, BHPC, NB, D], F16, tag="ctxs")

            # two persistent emT buffers (alternated by block parity) whose
            # invalid-triangle regions are zeroed once and never rewritten
            # (the in-loop copies are predicated on the band mask)
            emT_bufs = []
            for i in range(2):
                t = emtp.tile([W, 3 * W], F16, tag="emt", name=f"emT{i}")
                nc.vector.memset(t[:], 0.0)
                emT_bufs.append(t)

            qT = {}
            kT = {}

            def load_pair(p):
                qT[p] = qtp.tile([2 * D, S], F16, tag="qt", name=f"qT{p}")
                nc.sync.dma_start(qT[p][:], qt_i[p])
                kT[p] = ktp.tile([2 * D, S], F16, tag="kt", name=f"kT{p}")
                nc.sync.dma_start(kT[p][:], kt_i[p])

            for bh in range(BHPC):
                p, half = bh // 2, bh % 2
                if half == 0:
                    load_pair(p)
                dlo = half * D

                v_sb = vp.tile([W, NB, D], F16, tag="v")
                nc.sync.dma_start(
                    v_sb[:], v_i[bh].rearrange("(n x) d -> x n d", x=W)
                )

                for n in range(NB):
                    c_lo = 1 if n == 0 else 0
                    c_hi = 2 if n == NB - 1 else 3

                    psum_s = ps_s.tile([W, 3 * W], F32, tag="ps_s")
                    qk_args = (
                        psum_s[:, c_lo * W : c_hi * W],
                        qT[p][dlo : dlo + D, n * W : (n + 1) * W],
                        kT[p][dlo : dlo + D, (n - 1 + c_lo) * W : (n - 1 + c_hi) * W],
                    )
                    if c_hi - c_lo < 3:
                        # edge block: additive mask defines the columns the
                        # QK matmul does not cover
                        nc.tensor.matmul(
                            psum_s[:], ident[:], masks[:, 0 if n == 0 else 1, :],
                            start=True, stop=False,
                        )
                        nc.tensor.matmul(*qk_args, start=False, stop=True)
                    else:
                        nc.tensor.matmul(*qk_args, start=True, stop=True)

                    # exp writes the DRAM-bound rows directly (unmasked in
                    # the triangles; the host band extraction skips them)
                    em_m = emn_all[:, bh, n]
                    nc.scalar.activation(
                        em_m[:], psum_s[:], mybir.ActivationFunctionType.Exp,
                        bias=ebias[:], scale=SCALE,
                    )

                    # transpose valid chunks into one PSUM bank; one
                    # band-mask-predicated copy into a pre-zeroed buffer
                    pt = ps_t.tile([W, 3 * W], F16, tag="ps_t")
                    for c in range(c_lo, c_hi):
                        nc.tensor.transpose(
                            pt[:, c * W : (c + 1) * W],
                            em_m[:, c * W : (c + 1) * W], ident[:],
                        )
                    emT = emT_bufs[n % 2]
                    sl = slice(c_lo * W, c_hi * W)
                    nc.vector.copy_predicated(emT[:, sl], maskT[:, sl], pt[:, sl])

                    psum_ctx = ps_c.tile([W, D], F32, tag="ps_c")
                    for c in range(c_lo, c_hi):
                        nc.tensor.matmul(
                            psum_ctx[:], emT[:, c * W : (c + 1) * W],
                            v_sb[:, n - 1 + c, :],
                            start=(c == c_lo), stop=(c == c_hi - 1),
                        )
                    # unnormalized ctx; host divides by the band row sums
                    if n % 2 == 0:
                        nc.scalar.copy(ctx_all[:, bh, n, :], psum_ctx[:])
                    else:
                        nc.vector.tensor_copy(ctx_all[:, bh, n, :], psum_ctx[:])

                nc.sync.dma_start(
                    ctx_o[bh].rearrange("(n x) d -> x n d", x=W),
                    ctx_all[:, bh],
                )
                nc.sync.dma_start(
                    em_o[bh].rearrange("(n x) y -> x n y", x=W),
                    emn_all[:, bh],
                )

    nc.compile()
    return nc


_PROGRAM = None


def _get_program():
    global _PROGRAM
    if _PROGRAM is None:
        _PROGRAM = _build_program()
    return _PROGRAM


# host-side diagonal band extraction index: band[x, t] = em_row[x, x + t]
_BAND_IDX = (np.arange(W)[:, None] + np.arange(T)[None, :])  # [128, 257]


def _extract_band(em_raw):
    """[nbh, S, 384] fp16 block rows -> [nbh, S, 257] f32 band."""
    nbh, s, _ = em_raw.shape
    nb = s // W
    em4 = em_raw.reshape(nbh, nb, W, 3 * W)
    idx = _BAND_IDX[None, None].astype(np.intp)
    band = np.take_along_axis(em4, np.broadcast_to(idx, (nbh, nb, W, T)), axis=3)
    return band.reshape(nbh, s, T)


def _prep_core_inputs(qf, kf, vf, lo):
    """Host-side fp16 cast + [d, seq] transpose + bh-pair packing."""
    q16 = qf[lo : lo + BHPC].astype(np.float16)
    k16 = kf[lo : lo + BHPC].astype(np.float16)
    v16 = vf[lo : lo + BHPC].astype(np.float16)
    qt = np.zeros((NPAIR, 2 * D, S), np.float16)
    kt = np.zeros((NPAIR, 2 * D, S), np.float16)
    for bh in range(BHPC):
        p, half = bh // 2, bh % 2
        qt[p, half * D : (half + 1) * D] = q16[bh].T
        kt[p, half * D : (half + 1) * D] = k16[bh].T
    return {
        "qt": np.ascontiguousarray(qt),
        "kt": np.ascontiguousarray(kt),
        "v": np.ascontiguousarray(v16),
    }


def kernel(q, k, v, numeric_embedding_manager=None, **_unused):
    nc = _get_program()
    qf = np.asarray(q, dtype=np.float32).reshape(B * H, S, D)
    kf = np.asarray(k, dtype=np.float32).reshape(B * H, S, D)
    vf = np.asarray(v, dtype=np.float32).reshape(B * H, S, D)

    in_maps = [
        _prep_core_inputs(qf, kf, vf, i * BHPC) for i in range(NCORES)
    ]
    res = run_bass_kernel_spmd(nc, in_maps, core_ids=list(range(NCORES)))

    ctx_raw = np.concatenate(
        [res.results[i]["ctx"] for i in range(NCORES)], axis=0
    ).astype(np.float32)
    em_raw = np.concatenate(
        [res.results[i]["emn"] for i in range(NCORES)], axis=0
    )
    band = _extract_band(em_raw).astype(np.float32)
    rn = 1.0 / band.sum(axis=2, keepdims=True)
    probs = band * rn
    ctx = (ctx_raw * rn).reshape(B, H, S, D)
    return ctx, probs


# revision 30
# speedup vs baseline: 1.4457x; 1.1099x over previous
"""Longformer sliding-window attention (W=128) on 8 Trainium2 NeuronCores.

Strategy (fp16 compute, f32 softmax stats, batch*head sharding):
  - 24 (b,h) slices across 8 cores, 3 per core; window attention is local.
  - Host pre-casts q/k/v to fp16 and pre-transposes q/k to [d, seq] (with
    two bh slices packed along d to fill 128 SBUF partitions), so the
    device sees DMA-friendly layouts and half the input bytes.
  - Per (bh, query block n of 128):
      PE : mask matmul (identity @ maskbias, PSUM start) then QK matmul
           (lhsT=qT[64,128] fp16, rhs=kT[64,384] fp16) accumulating into
           the same PSUM bank -> masked scores f32
      ACT: em = exp(scale*scores - 6) fp16 (uniform -6 shift cancels in
           softmax, keeps exp in fp16 range), accum_out = row sums f32
      DVE: r = 1/sums;  em_n = em * r  -> normalized prob rows [128, 384]
      PE : transpose em_n 128-chunks -> PSUM, DVE copy -> SBUF, then
           ctx[128,64] = sum_c emT_c.T @ v_c accumulated in PSUM f32
  - Device writes em_n blocks [128, NB, 384] fp16 per bh; the host
    extracts the diagonal 257-band (probs[x, t] = em_n[n, x, x+t]) and
    upcasts to f32. End-to-end abs-rel error ~5e-4 vs the f32 reference.
"""

import numpy as np

import concourse.bacc as bacc
import concourse.mybir as mybir
import concourse.tile as tile
from concourse.bass_utils import run_bass_kernel_spmd

F16 = mybir.dt.float16
F32 = mybir.dt.float32
U8 = mybir.dt.uint8
U16 = mybir.dt.uint16

NCORES = 8
B, H, S, D = 2, 12, 4096, 64
W = 128
NB = S // W          # 32 query blocks per bh
T = 2 * W + 1        # 257 band width
BHPC = (B * H) // NCORES  # 3 bh slices per core
NPAIR = (BHPC + 1) // 2   # bh pairs packed along d into 128 partitions
NEG = -30720.0
SCALE = 0.125        # 1/sqrt(D)
EBIAS = -6.0         # uniform exp shift, cancels in softmax


def _np_constants():
    x = np.arange(W)[:, None]
    y = np.arange(3 * W)[None, :]
    band = (y - x >= 0) & (y - x <= 2 * W)
    first = band & (y >= W)
    last = band & (y < 2 * W)
    # additive -30720 masks for the two edge blocks (define full PSUM rows)
    masks = np.stack(
        [np.where(m, 0.0, NEG) for m in (first, last)]
    ).astype(np.float16)  # [2, 128, 384]
    # transposed band-validity mask for the predicated emT copies:
    # emT layout is [y_local, c*W + x] = em[x, c*W + y_local]
    yl = np.arange(W)[:, None]
    xx = np.arange(W)[None, :]
    mt = np.zeros((W, 3 * W), np.uint16)
    for c in range(3):
        yg = c * W + yl
        mt[:, c * W : (c + 1) * W] = ((yg - xx >= 0) & (yg - xx <= 2 * W))
    ident = np.eye(W, dtype=np.float16)
    ebias = np.full((W, 1), EBIAS, dtype=np.float32)
    return masks, mt, ident, ebias


def _build_program(BHPC=BHPC, NB=NB):
    S = NB * W
    NPAIR = (BHPC + 1) // 2
    nc = bacc.Bacc("TRN2", target_bir_lowering=False, debug=False)

    # host-prepared fp16 inputs: qt/kt are [d, seq] transposed, two bh
    # packed along d; v is in natural [seq, d] layout
    qt_i = nc.dram_tensor("qt", [NPAIR, 2 * D, S], F16, kind="ExternalInput").ap()
    kt_i = nc.dram_tensor("kt", [NPAIR, 2 * D, S], F16, kind="ExternalInput").ap()
    v_i = nc.dram_tensor("v", [BHPC, S, D], F16, kind="ExternalInput").ap()
    # unnormalized outputs; host extracts the 257 diagonal band, computes
    # row sums over it, and normalizes both probs and ctx
    ctx_o = nc.dram_tensor("ctx", [BHPC, S, D], F16, kind="ExternalOutput").ap()
    em_o = nc.dram_tensor("emn", [BHPC, S, 3 * W], F16, kind="ExternalOutput").ap()

    masks_np, mt_np, ident_np, ebias_np = _np_constants()
    masks_d = nc.inline_tensor(masks_np, "masks_c").ap()
    mt_d = nc.inline_tensor(mt_np, "mt_c").ap()
    ident_d = nc.inline_tensor(ident_np, "ident_c").ap()
    ebias_d = nc.inline_tensor(ebias_np, "ebias_c").ap()

    with tile.TileContext(nc) as tc:
        with (
            tc.tile_pool(name="const", bufs=1) as constp,
            tc.tile_pool(name="qt", bufs=2) as qtp,
            tc.tile_pool(name="kt", bufs=2) as ktp,
            tc.tile_pool(name="vp", bufs=2) as vp,
            tc.tile_pool(name="emn", bufs=1) as emnp,
            tc.tile_pool(name="emt", bufs=2) as emtp,
            tc.tile_pool(name="ctxs", bufs=1) as ctxsp,
            tc.tile_pool(name="ps_s", bufs=3, space="PSUM") as ps_s,
            tc.tile_pool(name="ps_t", bufs=3, space="PSUM") as ps_t,
            tc.tile_pool(name="ps_c", bufs=2, space="PSUM") as ps_c,
        ):
            masks = constp.tile([W, 2, 3 * W], F16, tag="masks")
            nc.sync.dma_start(masks[:], masks_d.rearrange("m x y -> x m y"))
            maskT = constp.tile([W, 3 * W], U16, tag="maskT")
            nc.sync.dma_start(maskT[:], mt_d)
            ident = constp.tile([W, W], F16, tag="ident")
            nc.sync.dma_start(ident[:], ident_d)
            ebias = constp.tile([W, 1], F32, tag="ebias")
            nc.sync.dma_start(ebias[:], ebias_d)

            emn_all = emnp.tile([W, BHPC, NB, 3 * W], F16, tag="emn")
            ctx_all = ctxsp.tile([W, BHPC, NB, D], F16, tag="ctxs")

            # two persistent emT buffers (alternated by block parity) whose
            # invalid-triangle regions are zeroed once and never rewritten
            # (the in-loop copies are predicated on the band mask)
            emT_bufs = []
            for i in range(2):
                t = emtp.tile([W, 3 * W], F16, tag="emt", name=f"emT{i}")
                nc.vector.memset(t[:], 0.0)
                emT_bufs.append(t)

            qT = {}
            kT = {}

            def load_pair(p):
                qT[p] = qtp.tile([2 * D, S], F16, tag="qt", name=f"qT{p}")
                nc.sync.dma_start(qT[p][:], qt_i[p])
                kT[p] = ktp.tile([2 * D, S], F16, tag="kt", name=f"kT{p}")
                nc.sync.dma_start(kT[p][:], kt_i[p])

            for bh in range(BHPC):
                p, half = bh // 2, bh % 2
                if half == 0:
                    load_pair(p)
                dlo = half * D

                v_sb = vp.tile([W, NB, D], F16, tag="v")
                nc.sync.dma_start(
                    v_sb[:], v_i[bh].rearrange("(n x) d -> x n d", x=W)
                )

                for n in range(NB):
                    c_lo = 1 if n == 0 else 0
                    c_hi = 2 if n == NB - 1 else 3

                    psum_s = ps_s.tile([W, 3 * W], F32, tag="ps_s")
                    qk_args = (
                        psum_s[:, c_lo * W : c_hi * W],
                        qT[p][dlo : dlo + D, n * W : (n + 1) * W],
                        kT[p][dlo : dlo + D, (n - 1 + c_lo) * W : (n - 1 + c_hi) * W],
                    )
                    if c_hi - c_lo < 3:
                        # edge block: additive mask defines the columns the
                        # QK matmul does not cover
                        nc.tensor.matmul(
                            psum_s[:], ident[:], masks[:, 0 if n == 0 else 1, :],
                            start=True, stop=False,
                        )
                        nc.tensor.matmul(*qk_args, start=False, stop=True)
                    else:
                        nc.tensor.matmul(*qk_args, start=True, stop=True)

                    # exp writes the DRAM-bound rows directly (unmasked in
                    # the triangles; the host band extraction skips them)
                    em_m = emn_all[:, bh, n]
                    nc.scalar.activation(
                        em_m[:], psum_s[:], mybir.ActivationFunctionType.Exp,
                        bias=ebias[:], scale=SCALE,
                    )

                    # transpose valid chunks into one PSUM bank; one
                    # band-mask-predicated copy into a pre-zeroed buffer
                    pt = ps_t.tile([W, 3 * W], F16, tag="ps_t")
                    for c in range(c_lo, c_hi):
                        nc.tensor.transpose(
                            pt[:, c * W : (c + 1) * W],
                            em_m[:, c * W : (c + 1) * W], ident[:],
                        )
                    emT = emT_bufs[n % 2]
                    sl = slice(c_lo * W, c_hi * W)
                    nc.vector.copy_predicated(emT[:, sl], maskT[:, sl], pt[:, sl])

                    psum_ctx = ps_c.tile([W, D], F32, tag="ps_c")
                    for c in range(c_lo, c_hi):
                        nc.tensor.matmul(
                            psum_ctx[:], emT[:, c * W : (c + 1) * W],
                            v_sb[:, n - 1 + c, :],
                            start=(c == c_lo), stop=(c == c_hi - 1),
                        )
                    # unnormalized ctx; host divides by the band row sums
                    nc.vector.tensor_copy(ctx_all[:, bh, n, :], psum_ctx[:])

                for hf in range(2):
                    hn = NB // 2
                    nc.sync.dma_start(
                        ctx_o[bh].rearrange("(n x) d -> x n d", x=W)[
                            :, hf * hn : (hf + 1) * hn
                        ],
                        ctx_all[:, bh, hf * hn : (hf + 1) * hn],
                    )
                    nc.sync.dma_start(
                        em_o[bh].rearrange("(n x) y -> x n y", x=W)[
                            :, hf * hn : (hf + 1) * hn
                        ],
                        emn_all[:, bh, hf * hn : (hf + 1) * hn],
                    )

    nc.compile()
    return nc


_PROGRAM = None


def _get_program():
    global _PROGRAM
    if _PROGRAM is None:
        _PROGRAM = _build_program()
    return _PROGRAM


# host-side diagonal band extraction index: band[x, t] = em_row[x, x + t]
_BAND_IDX = (np.arange(W)[:, None] + np.arange(T)[None, :])  # [128, 257]


def _extract_band(em_raw):
    """[nbh, S, 384] fp16 block rows -> [nbh, S, 257] f32 band."""
    nbh, s, _ = em_raw.shape
    nb = s // W
    em4 = em_raw.reshape(nbh, nb, W, 3 * W)
    idx = _BAND_IDX[None, None].astype(np.intp)
    band = np.take_along_axis(em4, np.broadcast_to(idx, (nbh, nb, W, T)), axis=3)
    return band.reshape(nbh, s, T)


def _prep_core_inputs(qf, kf, vf, lo):
    """Host-side fp16 cast + [d, seq] transpose + bh-pair packing."""
    q16 = qf[lo : lo + BHPC].astype(np.float16)
    k16 = kf[lo : lo + BHPC].astype(np.float16)
    v16 = vf[lo : lo + BHPC].astype(np.float16)
    qt = np.zeros((NPAIR, 2 * D, S), np.float16)
    kt = np.zeros((NPAIR, 2 * D, S), np.float16)
    for bh in range(BHPC):
        p, half = bh // 2, bh % 2
        qt[p, half * D : (half + 1) * D] = q16[bh].T
        kt[p, half * D : (half + 1) * D] = k16[bh].T
    return {
        "qt": np.ascontiguousarray(qt),
        "kt": np.ascontiguousarray(kt),
        "v": np.ascontiguousarray(v16),
    }


def kernel(q, k, v, numeric_embedding_manager=None, **_unused):
    nc = _get_program()
    qf = np.asarray(q, dtype=np.float32).reshape(B * H, S, D)
    kf = np.asarray(k, dtype=np.float32).reshape(B * H, S, D)
    vf = np.asarray(v, dtype=np.float32).reshape(B * H, S, D)

    in_maps = [
        _prep_core_inputs(qf, kf, vf, i * BHPC) for i in range(NCORES)
    ]
    res = run_bass_kernel_spmd(nc, in_maps, core_ids=list(range(NCORES)))

    ctx_raw = np.concatenate(
        [res.results[i]["ctx"] for i in range(NCORES)], axis=0
    ).astype(np.float32)
    em_raw = np.concatenate(
        [res.results[i]["emn"] for i in range(NCORES)], axis=0
    )
    band = _extract_band(em_raw).astype(np.float32)
    rn = 1.0 / band.sum(axis=2, keepdims=True)
    probs = band * rn
    ctx = (ctx_raw * rn).reshape(B, H, S, D)
    return ctx, probs


# revision 33
# speedup vs baseline: 1.7760x; 1.2285x over previous
"""Longformer sliding-window attention (W=128) on 8 Trainium2 NeuronCores.

Strategy (fp16 compute, f32 softmax stats, batch*head sharding):
  - 24 (b,h) slices across 8 cores, 3 per core; window attention is local.
  - Host pre-casts q/k/v to fp16 and pre-transposes q/k to [d, seq] (with
    two bh slices packed along d to fill 128 SBUF partitions), so the
    device sees DMA-friendly layouts and half the input bytes.
  - Per (bh, query block n of 128):
      PE : mask matmul (identity @ maskbias, PSUM start) then QK matmul
           (lhsT=qT[64,128] fp16, rhs=kT[64,384] fp16) accumulating into
           the same PSUM bank -> masked scores f32
      ACT: em = exp(scale*scores - 6) fp16 (uniform -6 shift cancels in
           softmax, keeps exp in fp16 range), accum_out = row sums f32
      DVE: r = 1/sums;  em_n = em * r  -> normalized prob rows [128, 384]
      PE : transpose em_n 128-chunks -> PSUM, DVE copy -> SBUF, then
           ctx[128,64] = sum_c emT_c.T @ v_c accumulated in PSUM f32
  - Device writes em_n blocks [128, NB, 384] fp16 per bh; the host
    extracts the diagonal 257-band (probs[x, t] = em_n[n, x, x+t]) and
    upcasts to f32. End-to-end abs-rel error ~5e-4 vs the f32 reference.
"""

import numpy as np

import concourse.bacc as bacc
import concourse.mybir as mybir
import concourse.tile as tile
from concourse.bass_utils import run_bass_kernel_spmd

F16 = mybir.dt.float16
F32 = mybir.dt.float32
U8 = mybir.dt.uint8
U16 = mybir.dt.uint16

NCORES = 8
B, H, S, D = 2, 12, 4096, 64
W = 128
NB = S // W          # 32 query blocks per bh
T = 2 * W + 1        # 257 band width
BHPC = (B * H) // NCORES  # 3 bh slices per core
NPAIR = (BHPC + 1) // 2   # bh pairs packed along d into 128 partitions
NEG = -30720.0
SCALE = 0.125        # 1/sqrt(D)
EBIAS = -6.0         # uniform exp shift, cancels in softmax


def _np_constants():
    x = np.arange(W)[:, None]
    y = np.arange(3 * W)[None, :]
    band = (y - x >= 0) & (y - x <= 2 * W)
    first = band & (y >= W)
    last = band & (y < 2 * W)
    # additive -30720 masks for the two edge blocks (define full PSUM rows)
    masks = np.stack(
        [np.where(m, 0.0, NEG) for m in (first, last)]
    ).astype(np.float16)  # [2, 128, 384]
    # transposed band-validity mask for the predicated emT copies:
    # emT layout is [y_local, c*W + x] = em[x, c*W + y_local]
    yl = np.arange(W)[:, None]
    xx = np.arange(W)[None, :]
    mt = np.zeros((W, 3 * W), np.uint16)
    for c in range(3):
        yg = c * W + yl
        mt[:, c * W : (c + 1) * W] = ((yg - xx >= 0) & (yg - xx <= 2 * W))
    ident = np.eye(W, dtype=np.float16)
    ebias = np.full((W, 1), EBIAS, dtype=np.float32)
    return masks, mt, ident, ebias


def _build_program(BHPC=BHPC, NB=NB):
    S = NB * W
    NPAIR = (BHPC + 1) // 2
    nc = bacc.Bacc("TRN2", target_bir_lowering=False, debug=False)

    # host-prepared fp16 inputs: qt/kt are [d, seq] transposed, two bh
    # packed along d; v is in natural [seq, d] layout
    qt_i = nc.dram_tensor("qt", [NPAIR, 2 * D, S], F16, kind="ExternalInput").ap()
    kt_i = nc.dram_tensor("kt", [NPAIR, 2 * D, S], F16, kind="ExternalInput").ap()
    v_i = nc.dram_tensor("v", [BHPC, S, D], F16, kind="ExternalInput").ap()
    # unnormalized outputs; host extracts the 257 diagonal band, computes
    # row sums over it, and normalizes both probs and ctx
    ctx_o = nc.dram_tensor("ctx", [BHPC, S, D], F16, kind="ExternalOutput").ap()
    em_o = nc.dram_tensor("emn", [BHPC, S, 3 * W], F16, kind="ExternalOutput").ap()

    masks_np, mt_np, ident_np, ebias_np = _np_constants()
    masks_d = nc.inline_tensor(masks_np, "masks_c").ap()
    mt_d = nc.inline_tensor(mt_np, "mt_c").ap()
    ident_d = nc.inline_tensor(ident_np, "ident_c").ap()
    ebias_d = nc.inline_tensor(ebias_np, "ebias_c").ap()

    with tile.TileContext(nc) as tc:
        with (
            tc.tile_pool(name="const", bufs=1) as constp,
            tc.tile_pool(name="qt", bufs=2) as qtp,
            tc.tile_pool(name="kt", bufs=2) as ktp,
            tc.tile_pool(name="vp", bufs=3) as vp,
            tc.tile_pool(name="emn", bufs=1) as emnp,
            tc.tile_pool(name="emt", bufs=4) as emtp,
            tc.tile_pool(name="ctxs", bufs=1) as ctxsp,
            tc.tile_pool(name="ps_s", bufs=3, space="PSUM") as ps_s,
            tc.tile_pool(name="ps_t", bufs=3, space="PSUM") as ps_t,
            tc.tile_pool(name="ps_c", bufs=2, space="PSUM") as ps_c,
        ):
            masks = constp.tile([W, 2, 3 * W], F16, tag="masks")
            nc.sync.dma_start(masks[:], masks_d.rearrange("m x y -> x m y"))
            maskT = constp.tile([W, 3 * W], U16, tag="maskT")
            nc.sync.dma_start(maskT[:], mt_d)
            ident = constp.tile([W, W], F16, tag="ident")
            nc.sync.dma_start(ident[:], ident_d)
            ebias = constp.tile([W, 1], F32, tag="ebias")
            nc.sync.dma_start(ebias[:], ebias_d)

            emn_all = emnp.tile([W, BHPC, NB, 3 * W], F16, tag="emn")
            ctx_all = ctxsp.tile([W, BHPC, NB, D], F16, tag="ctxs")

            # two persistent emT buffers (alternated by block parity) whose
            # invalid-triangle regions are zeroed once and never rewritten
            # (the in-loop copies are predicated on the band mask)
            emT_bufs = []
            for i in range(4):
                t = emtp.tile([W, 3 * W], F16, tag="emt", name=f"emT{i}")
                nc.vector.memset(t[:], 0.0)
                emT_bufs.append(t)

            qT = {}
            kT = {}

            def load_pair(p):
                qT[p] = qtp.tile([2 * D, S], F16, tag="qt", name=f"qT{p}")
                nc.sync.dma_start(qT[p][:], qt_i[p])
                kT[p] = ktp.tile([2 * D, S], F16, tag="kt", name=f"kT{p}")
                nc.sync.dma_start(kT[p][:], kt_i[p])

            groups = [
                tuple(bh for bh in range(2 * p, min(2 * p + 2, BHPC)))
                for p in range(NPAIR)
            ]
            for p, group in enumerate(groups):
                load_pair(p)
                v_sb = {}
                for bh in group:
                    v_sb[bh] = vp.tile(
                        [W, NB, D], F16, tag="v", name=f"v{bh}"
                    )
                    nc.sync.dma_start(
                        v_sb[bh][:], v_i[bh].rearrange("(n x) d -> x n d", x=W)
                    )

                for n in range(NB):
                    c_lo = 1 if n == 0 else 0
                    c_hi = 2 if n == NB - 1 else 3
                    sl = slice(c_lo * W, c_hi * W)

                    # QK matmuls for the paired bh slices are emitted
                    # back-to-back: they use disjoint PE row groups (rows
                    # 0-63 / 64-127) and run concurrently on the array
                    psum_s = {}
                    for i, bh in enumerate(group):
                        dlo = i * D
                        psum_s[bh] = ps_s.tile(
                            [W, 3 * W], F32, tag="ps_s", name=f"ps_s{bh}"
                        )
                        qk_args = (
                            psum_s[bh][:, sl],
                            qT[p][dlo : dlo + D, n * W : (n + 1) * W],
                            kT[p][dlo : dlo + D,
                                  (n - 1 + c_lo) * W : (n - 1 + c_hi) * W],
                        )
                        if c_hi - c_lo < 3:
                            # edge block: additive mask defines the columns
                            # the QK matmul does not cover
                            nc.tensor.matmul(
                                psum_s[bh][:], ident[:],
                                masks[:, 0 if n == 0 else 1, :],
                                start=True, stop=False,
                            )
                            nc.tensor.matmul(*qk_args, start=False, stop=True)
                        else:
                            nc.tensor.matmul(*qk_args, start=True, stop=True)

                    for i, bh in enumerate(group):
                        # exp writes the DRAM-bound rows directly (unmasked
                        # in the triangles; host band extraction skips them)
                        em_m = emn_all[:, bh, n]
                        nc.scalar.activation(
                            em_m[:], psum_s[bh][:],
                            mybir.ActivationFunctionType.Exp,
                            bias=ebias[:], scale=SCALE,
                        )

                        # transpose valid chunks into one PSUM bank; one
                        # band-mask-predicated copy into a pre-zeroed buffer
                        pt = ps_t.tile([W, 3 * W], F16, tag="ps_t", name="pt")
                        for c in range(c_lo, c_hi):
                            nc.tensor.transpose(
                                pt[:, c * W : (c + 1) * W],
                                em_m[:, c * W : (c + 1) * W], ident[:],
                            )
                        emT = emT_bufs[(2 * n + i) % len(emT_bufs)]
                        nc.vector.copy_predicated(
                            emT[:, sl], maskT[:, sl], pt[:, sl]
                        )

                        psum_ctx = ps_c.tile([W, D], F32, tag="ps_c", name="pc")
                        for c in range(c_lo, c_hi):
                            nc.tensor.matmul(
                                psum_ctx[:], emT[:, c * W : (c + 1) * W],
                                v_sb[bh][:, n - 1 + c, :],
                                start=(c == c_lo), stop=(c == c_hi - 1),
                            )
                        # unnormalized ctx; host divides by band row sums
                        nc.vector.tensor_copy(
                            ctx_all[:, bh, n, :], psum_ctx[:]
                        )

                for bh in group:
                    for hf in range(2):
                        hn = NB // 2
                        nc.sync.dma_start(
                            ctx_o[bh].rearrange("(n x) d -> x n d", x=W)[
                                :, hf * hn : (hf + 1) * hn
                            ],
                            ctx_all[:, bh, hf * hn : (hf + 1) * hn],
                        )
                        nc.sync.dma_start(
                            em_o[bh].rearrange("(n x) y -> x n y", x=W)[
                                :, hf * hn : (hf + 1) * hn
                            ],
                            emn_all[:, bh, hf * hn : (hf + 1) * hn],
                        )

    nc.compile()
    return nc


_PROGRAM = None


def _get_program():
    global _PROGRAM
    if _PROGRAM is None:
        _PROGRAM = _build_program()
    return _PROGRAM


# host-side diagonal band extraction index: band[x, t] = em_row[x, x + t]
_BAND_IDX = (np.arange(W)[:, None] + np.arange(T)[None, :])  # [128, 257]


def _extract_band(em_raw):
    """[nbh, S, 384] fp16 block rows -> [nbh, S, 257] f32 band."""
    nbh, s, _ = em_raw.shape
    nb = s // W
    em4 = em_raw.reshape(nbh, nb, W, 3 * W)
    idx = _BAND_IDX[None, None].astype(np.intp)
    band = np.take_along_axis(em4, np.broadcast_to(idx, (nbh, nb, W, T)), axis=3)
    return band.reshape(nbh, s, T)


def _prep_core_inputs(qf, kf, vf, lo):
    """Host-side fp16 cast + [d, seq] transpose + bh-pair packing."""
    q16 = qf[lo : lo + BHPC].astype(np.float16)
    k16 = kf[lo : lo + BHPC].astype(np.float16)
    v16 = vf[lo : lo + BHPC].astype(np.float16)
    qt = np.zeros((NPAIR, 2 * D, S), np.float16)
    kt = np.zeros((NPAIR, 2 * D, S), np.float16)
    for bh in range(BHPC):
        p, half = bh // 2, bh % 2
        qt[p, half * D : (half + 1) * D] = q16[bh].T
        kt[p, half * D : (half + 1) * D] = k16[bh].T
    return {
        "qt": np.ascontiguousarray(qt),
        "kt": np.ascontiguousarray(kt),
        "v": np.ascontiguousarray(v16),
    }


def kernel(q, k, v, numeric_embedding_manager=None, **_unused):
    nc = _get_program()
    qf = np.asarray(q, dtype=np.float32).reshape(B * H, S, D)
    kf = np.asarray(k, dtype=np.float32).reshape(B * H, S, D)
    vf = np.asarray(v, dtype=np.float32).reshape(B * H, S, D)

    in_maps = [
        _prep_core_inputs(qf, kf, vf, i * BHPC) for i in range(NCORES)
    ]
    res = run_bass_kernel_spmd(nc, in_maps, core_ids=list(range(NCORES)))

    ctx_raw = np.concatenate(
        [res.results[i]["ctx"] for i in range(NCORES)], axis=0
    ).astype(np.float32)
    em_raw = np.concatenate(
        [res.results[i]["emn"] for i in range(NCORES)], axis=0
    )
    band = _extract_band(em_raw).astype(np.float32)
    rn = 1.0 / band.sum(axis=2, keepdims=True)
    probs = band * rn
    ctx = (ctx_raw * rn).reshape(B, H, S, D)
    return ctx, probs


# revision 35
# speedup vs baseline: 2.0597x; 1.1597x over previous
"""Longformer sliding-window attention (W=128) on 8 Trainium2 NeuronCores.

Strategy (fp16 compute, f32 scores, batch*head sharding):
  - 24 (b,h) slices across 8 cores, 3 per core; window attention is local
    per slice, so no collectives.
  - Host pre-casts q/k/v to fp16, pre-transposes q/k to [d, seq] with two
    bh slices packed along d (fills the 128 SBUF partitions), and lays v
    out in the device's [x, block, d] tiling, so every DMA is a plain
    contiguous copy with multi-KB descriptors.
  - Per query block n (128 queries), both packed bh slices interleaved:
      PE : QK matmuls for the two bh emitted back-to-back - they occupy
           disjoint row groups (rows 0-63 / 64-127) and run concurrently;
           scores land in one paired PSUM tile [128, 2, 512] f32.
           Edge blocks add a -30720 additive mask matmul (identity @ mask)
           so out-of-range slots become exp(..) = 0.
      ACT: one exp over both slices: em = exp(s/8 - 6) -> fp16 rows,
           written straight into the DRAM-bound staging tile.
      PE : transpose the 3 valid 128-chunks of each em row block -> PSUM
           (fp16), then one band-mask-predicated DVE copy into pre-zeroed
           SBUF buffers (kills the out-of-window triangles for free).
      PE : ctx[128, 2, 64] += emT_c.T @ v_c (3 accumulating matmuls per
           slice); one DVE cast copies both ctx slices out.
  - Outputs are unnormalized (em rows + ctx); the host extracts the 257
    diagonal band (probs[x,t] = em[n, x, x+t]), computes row sums over the
    band, and normalizes probs and ctx. End-to-end abs-rel error ~1e-3.
"""

import numpy as np

import concourse.bacc as bacc
import concourse.mybir as mybir
import concourse.tile as tile
from concourse.bass_utils import run_bass_kernel_spmd

F16 = mybir.dt.float16
F32 = mybir.dt.float32
U16 = mybir.dt.uint16

NCORES = 8
B, H, S, D = 2, 12, 4096, 64
W = 128
NB = S // W          # 32 query blocks per bh
T = 2 * W + 1        # 257 band width
BHPC = (B * H) // NCORES  # 3 bh slices per core
NPAIR = (BHPC + 1) // 2   # bh pairs packed along d into 128 partitions
NEG = -30720.0
SCALE = 0.125        # 1/sqrt(D)
EBIAS = -6.0         # uniform exp shift, cancels in softmax
PS = 512             # paired-PSUM per-slice pitch (bank aligned)


def _np_constants():
    x = np.arange(W)[:, None]
    y = np.arange(3 * W)[None, :]
    band = (y - x >= 0) & (y - x <= 2 * W)
    first = band & (y >= W)
    last = band & (y < 2 * W)
    # additive -30720 masks for the two edge blocks (define full PSUM rows)
    masks = np.stack(
        [np.where(m, 0.0, NEG) for m in (first, last)]
    ).astype(np.float16)  # [2, 128, 384]
    # transposed band-validity mask for the predicated emT copies:
    # emT layout is [y_local, c*W + x] = em[x, c*W + y_local]
    yl = np.arange(W)[:, None]
    xx = np.arange(W)[None, :]
    mt = np.zeros((W, 3 * W), np.uint16)
    for c in range(3):
        yg = c * W + yl
        mt[:, c * W : (c + 1) * W] = (yg - xx >= 0) & (yg - xx <= 2 * W)
    ident = np.eye(W, dtype=np.float16)
    ebias = np.full((W, 1), EBIAS, dtype=np.float32)
    return masks, mt, ident, ebias


def _build_program(BHPC=BHPC, NB=NB):
    S = NB * W
    NPAIR = (BHPC + 1) // 2
    nc = bacc.Bacc("TRN2", target_bir_lowering=False, debug=False)

    # host-prepared fp16 inputs (see _prep_core_inputs for layouts)
    qt_i = nc.dram_tensor("qt", [NPAIR, 2 * D, S], F16, kind="ExternalInput").ap()
    kt_i = nc.dram_tensor("kt", [NPAIR, 2 * D, S], F16, kind="ExternalInput").ap()
    v_i = nc.dram_tensor("v", [BHPC, W, NB, D], F16, kind="ExternalInput").ap()
    # unnormalized outputs in device tiling; host reorders + normalizes
    ctx_o = nc.dram_tensor("ctx", [BHPC, W, NB, D], F16, kind="ExternalOutput").ap()
    em_o = nc.dram_tensor("emn", [BHPC, W, NB, 3 * W], F16, kind="ExternalOutput").ap()

    masks_np, mt_np, ident_np, ebias_np = _np_constants()
    masks_d = nc.inline_tensor(masks_np, "masks_c").ap()
    mt_d = nc.inline_tensor(mt_np, "mt_c").ap()
    ident_d = nc.inline_tensor(ident_np, "ident_c").ap()
    ebias_d = nc.inline_tensor(ebias_np, "ebias_c").ap()

    with tile.TileContext(nc) as tc:
        with (
            tc.tile_pool(name="const", bufs=1) as constp,
            tc.tile_pool(name="qt", bufs=2) as qtp,
            tc.tile_pool(name="kt", bufs=2) as ktp,
            tc.tile_pool(name="vp", bufs=3) as vp,
            tc.tile_pool(name="emn", bufs=1) as emnp,
            tc.tile_pool(name="emt", bufs=2) as emtp,
            tc.tile_pool(name="ctxs", bufs=1) as ctxsp,
            tc.tile_pool(name="ps_s", bufs=2, space="PSUM") as ps_s,
            tc.tile_pool(name="ps_t", bufs=2, space="PSUM") as ps_t,
            tc.tile_pool(name="ps_c", bufs=2, space="PSUM") as ps_c,
        ):
            masks = constp.tile([W, 2, 3 * W], F16, tag="masks")
            nc.sync.dma_start(masks[:], masks_d.rearrange("m x y -> x m y"))
            maskT = constp.tile([W, 3 * W], U16, tag="maskT")
            nc.sync.dma_start(maskT[:], mt_d)
            ident = constp.tile([W, W], F16, tag="ident")
            nc.sync.dma_start(ident[:], ident_d)
            ebias = constp.tile([W, 1], F32, tag="ebias")
            nc.sync.dma_start(ebias[:], ebias_d)

            emn_all = emnp.tile([W, BHPC, NB, 3 * W], F16, tag="emn")
            ctx_all = ctxsp.tile([W, BHPC, NB, D], F16, tag="ctxs")

            # paired emT buffers (alternated by block parity) whose
            # invalid-triangle regions are zeroed once and never rewritten
            # (the in-loop copies are predicated on the band mask)
            emT_bufs = []
            for i in range(2):
                t = emtp.tile([W, 2, PS], F16, tag="emt", name=f"emT{i}")
                nc.vector.memset(t[:], 0.0)
                emT_bufs.append(t)

            qT = {}
            kT = {}

            def load_pair(p):
                qT[p] = qtp.tile([2 * D, S], F16, tag="qt", name=f"qT{p}")
                nc.sync.dma_start(qT[p][:], qt_i[p])
                kT[p] = ktp.tile([2 * D, S], F16, tag="kt", name=f"kT{p}")
                nc.sync.dma_start(kT[p][:], kt_i[p])

            groups = [
                tuple(bh for bh in range(2 * p, min(2 * p + 2, BHPC)))
                for p in range(NPAIR)
            ]
            for p, group in enumerate(groups):
                g = len(group)
                bh0 = group[0]
                load_pair(p)
                v_sb = {}
                for bh in group:
                    v_sb[bh] = vp.tile([W, NB, D], F16, tag="v", name=f"v{bh}")
                    nc.sync.dma_start(v_sb[bh][:], v_i[bh])

                for n in range(NB):
                    c_lo = 1 if n == 0 else 0
                    c_hi = 2 if n == NB - 1 else 3
                    sl = slice(c_lo * W, c_hi * W)
                    ncols = (c_hi - c_lo) * W

                    # QK matmuls for the paired bh slices are emitted
                    # back-to-back: they use disjoint PE row groups (rows
                    # 0-63 / 64-127) and run concurrently on the array
                    psum_s = ps_s.tile([W, 2, PS], F32, tag="ps_s", name="pss")
                    for i, bh in enumerate(group):
                        dlo = i * D
                        qk_args = (
                            psum_s[:, i, sl],
                            qT[p][dlo : dlo + D, n * W : (n + 1) * W],
                            kT[p][dlo : dlo + D,
                                  (n - 1 + c_lo) * W : (n - 1 + c_hi) * W],
                        )
                        if c_hi - c_lo < 3:
                            # edge block: additive mask defines the columns
                            # the QK matmul does not cover
                            nc.tensor.matmul(
                                psum_s[:, i, : 3 * W], ident[:],
                                masks[:, 0 if n == 0 else 1, :],
                                start=True, stop=False,
                            )
                            nc.tensor.matmul(*qk_args, start=False, stop=True)
                        else:
                            nc.tensor.matmul(*qk_args, start=True, stop=True)

                    # one exp over both slices, written straight into the
                    # DRAM-bound staging rows (triangles unmasked; the host
                    # band extraction skips them)
                    em_pair = bass_pair_ap(emn_all, bh0, n, g)
                    nc.scalar.activation(
                        em_pair, psum_s[:, :g, : 3 * W],
                        mybir.ActivationFunctionType.Exp,
                        bias=ebias[:], scale=SCALE,
                    )

                    # transpose valid chunks of both slices into one PSUM
                    # bank; one band-mask-predicated copy into pre-zeroed
                    # SBUF buffers
                    pt = ps_t.tile([W, 2, PS], F16, tag="ps_t", name="pt")
                    for i, bh in enumerate(group):
                        for c in range(c_lo, c_hi):
                            nc.tensor.transpose(
                                pt[:, i, c * W : (c + 1) * W],
                                emn_all[:, bh, n, c * W : (c + 1) * W],
                                ident[:],
                            )
                    emT = emT_bufs[n % 2]
                    nc.vector.copy_predicated(
                        emT[:, :g, sl],
                        maskT[:, sl].unsqueeze(1).broadcast_to([W, g, ncols]),
                        pt[:, :g, sl],
                    )

                    psum_ctx = ps_c.tile([W, 2, D], F32, tag="ps_c", name="pc")
                    for i, bh in enumerate(group):
                        for c in range(c_lo, c_hi):
                            nc.tensor.matmul(
                                psum_ctx[:, i, :],
                                emT[:, i, c * W : (c + 1) * W],
                                v_sb[bh][:, n - 1 + c, :],
                                start=(c == c_lo), stop=(c == c_hi - 1),
                            )
                    # unnormalized ctx for both slices in one cast copy;
                    # host divides by the band row sums
                    nc.vector.tensor_copy(
                        bass_pair_ap(ctx_all, bh0, n, g), psum_ctx[:, :g, :]
                    )

                for bh in group:
                    for hf in range(2):
                        hn = NB // 2
                        hs = slice(hf * hn, (hf + 1) * hn)
                        nc.sync.dma_start(ctx_o[bh][:, hs], ctx_all[:, bh, hs])
                        nc.sync.dma_start(em_o[bh][:, hs], emn_all[:, bh, hs])

    nc.compile()
    return nc


def bass_pair_ap(big, bh0, n, g):
    """AP over slices (bh0, n) and (bh0+1, n) of a [W, BHPC, NB, F] tile."""
    sub = big[:, bh0 : bh0 + g, n]
    return sub


_PROGRAM = None


def _get_program():
    global _PROGRAM
    if _PROGRAM is None:
        _PROGRAM = _build_program()
    return _PROGRAM


# host-side diagonal band extraction index: band[x, t] = em_row[x, x + t]
_BAND_IDX = (np.arange(W)[:, None] + np.arange(T)[None, :])  # [128, 257]


def _extract_band(em_raw):
    """[nbh, W, NB, 384] fp16 device tiling -> [nbh, S, 257] band."""
    nbh = em_raw.shape[0]
    nb = em_raw.shape[2]
    idx = _BAND_IDX[:, None, :].astype(np.intp)  # [W, 1, T]
    band = np.take_along_axis(
        em_raw, np.broadcast_to(idx, (nbh, W, nb, T)), axis=3
    )  # [nbh, W, nb, T]
    return band.transpose(0, 2, 1, 3).reshape(nbh, nb * W, T)


def _prep_core_inputs(qf, kf, vf, lo):
    """Host-side fp16 cast + [d, seq] transpose + bh-pair packing."""
    q16 = qf[lo : lo + BHPC].astype(np.float16)
    k16 = kf[lo : lo + BHPC].astype(np.float16)
    v16 = vf[lo : lo + BHPC].astype(np.float16)
    qt = np.zeros((NPAIR, 2 * D, S), np.float16)
    kt = np.zeros((NPAIR, 2 * D, S), np.float16)
    for bh in range(BHPC):
        p, half = bh // 2, bh % 2
        qt[p, half * D : (half + 1) * D] = q16[bh].T
        kt[p, half * D : (half + 1) * D] = k16[bh].T
    vdev = np.ascontiguousarray(
        v16.reshape(BHPC, NB, W, D).transpose(0, 2, 1, 3)
    )
    return {
        "qt": np.ascontiguousarray(qt),
        "kt": np.ascontiguousarray(kt),
        "v": vdev,
    }


def kernel(q, k, v, numeric_embedding_manager=None, **_unused):
    nc = _get_program()
    qf = np.asarray(q, dtype=np.float32).reshape(B * H, S, D)
    kf = np.asarray(k, dtype=np.float32).reshape(B * H, S, D)
    vf = np.asarray(v, dtype=np.float32).reshape(B * H, S, D)

    in_maps = [
        _prep_core_inputs(qf, kf, vf, i * BHPC) for i in range(NCORES)
    ]
    res = run_bass_kernel_spmd(nc, in_maps, core_ids=list(range(NCORES)))

    ctx_raw = np.concatenate(
        [res.results[i]["ctx"] for i in range(NCORES)], axis=0
    ).astype(np.float32)  # [24, W, NB, D]
    ctx_raw = ctx_raw.transpose(0, 2, 1, 3).reshape(B * H, S, D)
    em_raw = np.concatenate(
        [res.results[i]["emn"] for i in range(NCORES)], axis=0
    )
    band = _extract_band(em_raw).astype(np.float32)
    rn = 1.0 / band.sum(axis=2, keepdims=True)
    probs = band * rn
    ctx = (ctx_raw * rn).reshape(B, H, S, D)
    return ctx, probs
